# revision 3
# baseline (speedup 1.0000x reference)
"""AttnPool3D Trainium2 kernel.

Reference computation (B=2, C=128, D=48, H=96, W=96, N = D*H*W = 442368):
    logits = einsum('bcdhw,c->bdhw', feat, w_attn) + 2.0*clip(mask, 0, 1)
    w = softmax(logits.reshape(B, -1), axis=-1)
    out = einsum('bcn,bn->bc', feat.reshape(B, C, -1), w)

Sharding: 8 cores = (batch b in 0..1) x (spatial quarter q in 0..3).
Each core processes feat[b, :, q*Ns:(q+1)*Ns] (Ns = 110592) in ONE pass over
HBM (single-pass softmax without a max subtraction; logits are bounded, a
constant bias -8 prevents overflow and cancels in the final normalization):

  per chunk of 2048 spatial columns (54 chunks):
    - DMA feat chunk A [128, 2048] fp32 (C on partitions)
    - Pool/DVE: cast A -> Ah fp16
    - PE (per 512-sub-chunk, accumulating into PSUM [128, 512]):
        X  = wh_rep^T @ Ah      (logits "high" part, broadcast to all parts)
        X += wl_rep^T @ Ah      (w = wh + wl fp16 split: w captured to 2^-22)
        X += ones2^T @ [mh; ml] (2*clip(mask) fp16-split rows, K=2 matmul)
    - ACT: Pb = exp(X - 8), accum_out -> s_chunk [128, 1]
    - DVE: stt junk = A * Pb (fp32 ALU), accum_out -> v_chunk [128, 1]
  reduce v/s over chunks -> out_vs [128, 2]

Host combines: out[b, c] = sum_q v / sum_q s (fp64; the -8 bias cancels).
"""
import sys

sys.path.insert(0, "/opt/trn_rl_repo")

import numpy as np

import concourse.bass as bass
import concourse.tile as tile
from concourse import mybir, bacc
from concourse.bass_utils import run_bass_kernel_spmd

B, C = 2, 128
N_FULL = 48 * 96 * 96          # 442368
N_CORES = 8
Q_PER_B = 4                    # spatial quarters per batch
NS = N_FULL // Q_PER_B         # 110592 per core
F_CHUNK = 2048                 # spatial columns per chunk
N_CHUNKS = NS // F_CHUNK       # 54
SUB = 512                      # matmul free dim (one PSUM bank fp32)
N_SUB = F_CHUNK // SUB         # 4
EXP_BIAS = -8.0
DVE_CAST_EVERY = 3             # every 3rd chunk casts fp32->fp16 on DVE

f32 = mybir.dt.float32
f16 = mybir.dt.float16

_CACHED = {}


def _build(bench_reps=None):
    """bench_reps=None -> production straight-line kernel.
    bench_reps=R -> same body wrapped in a For_i(R) repeat loop (for HW
    timing via wall-clock deltas between two R values)."""
    nc = bacc.Bacc("TRN2", target_bir_lowering=False, debug=False)

    feat_dram = nc.dram_tensor("feat", [C, NS], f32, kind="ExternalInput")
    mrows_dram = nc.dram_tensor("mrows", [2, NS], f16, kind="ExternalInput")
    whrep_dram = nc.dram_tensor("whrep", [C, 128], f16, kind="ExternalInput")
    wlrep_dram = nc.dram_tensor("wlrep", [C, 128], f16, kind="ExternalInput")
    out_dram = nc.dram_tensor("out_vs", [C, 2], f32, kind="ExternalOutput")

    with tile.TileContext(nc) as tc:
        with (
            tc.tile_pool(name="weights", bufs=1) as wpool,
            tc.tile_pool(name="feat", bufs=3) as apool,
            tc.tile_pool(name="feat16", bufs=3) as ahpool,
            tc.tile_pool(name="mask", bufs=4) as mpool,
            tc.tile_pool(name="prob", bufs=3) as ppool,
            tc.tile_pool(name="junk", bufs=2) as jpool,
            tc.tile_pool(name="accs", bufs=1) as accpool,
            tc.tile_pool(name="psum", bufs=2, space="PSUM") as psum,
        ):
            whrep = wpool.tile([C, 128], f16)
            wlrep = wpool.tile([C, 128], f16)
            nc.sync.dma_start(whrep[:], whrep_dram.ap())
            nc.sync.dma_start(wlrep[:], wlrep_dram.ap())
            ones2 = wpool.tile([2, 128], f16)
            nc.vector.memset(ones2[:], 1.0)
            bias_t = wpool.tile([C, 1], f32)
            nc.vector.memset(bias_t[:], EXP_BIAS)

            v_accs = accpool.tile([C, N_CHUNKS], f32)
            s_accs = accpool.tile([C, N_CHUNKS], f32)

            def emit_chunk(ci):
                a = apool.tile([C, F_CHUNK], f32, tag="a")
                nc.sync.dma_start(
                    a[:], feat_dram.ap()[:, ci * F_CHUNK:(ci + 1) * F_CHUNK])
                mrows = mpool.tile([2, F_CHUNK], f16, tag="mrows")
                nc.sync.dma_start(
                    mrows[:], mrows_dram.ap()[:, ci * F_CHUNK:(ci + 1) * F_CHUNK])

                ah = ahpool.tile([C, F_CHUNK], f16, tag="ah")
                if ci % DVE_CAST_EVERY == DVE_CAST_EVERY - 1:
                    nc.vector.tensor_copy(ah[:], a[:])
                else:
                    nc.gpsimd.tensor_copy(ah[:], a[:])

                x = psum.tile([C, F_CHUNK], f32, tag="x")
                # group same-weight matmuls to minimize weight switching
                for si in range(N_SUB):
                    nc.tensor.matmul(x[:, si * SUB:(si + 1) * SUB], whrep[:],
                                     ah[:, si * SUB:(si + 1) * SUB],
                                     start=True, stop=False)
                for si in range(N_SUB):
                    nc.tensor.matmul(x[:, si * SUB:(si + 1) * SUB], wlrep[:],
                                     ah[:, si * SUB:(si + 1) * SUB],
                                     start=False, stop=False)
                for si in range(N_SUB):
                    nc.tensor.matmul(x[:, si * SUB:(si + 1) * SUB], ones2[:],
                                     mrows[:, si * SUB:(si + 1) * SUB],
                                     start=False, stop=True)

                pb = ppool.tile([C, F_CHUNK], f32, tag="pb")
                nc.scalar.activation(
                    pb[:], x[:], mybir.ActivationFunctionType.Exp,
                    bias=bias_t[:], scale=1.0,
                    accum_out=s_accs[:, ci:ci + 1],
                )

                junk = jpool.tile([C, F_CHUNK], f32, tag="junk")
                nc.vector.scalar_tensor_tensor(
                    junk[:], a[:], 1.0, pb[:],
                    op0=mybir.AluOpType.mult, op1=mybir.AluOpType.mult,
                    accum_out=v_accs[:, ci:ci + 1],
                )

            if bench_reps is None:
                for ci in range(N_CHUNKS):
                    emit_chunk(ci)
            else:
                with tc.For_i(0, bench_reps, 1,
                              hint_engines=(mybir.EngineType.PE,)):
                    for ci in range(N_CHUNKS):
                        emit_chunk(ci)

            out_sb = accpool.tile([C, 2], f32)
            nc.vector.reduce_sum(out_sb[:, 0:1], v_accs[:], axis=mybir.AxisListType.X)
            nc.vector.reduce_sum(out_sb[:, 1:2], s_accs[:], axis=mybir.AxisListType.X)
            nc.sync.dma_start(out_dram.ap(), out_sb[:])

    nc.compile()
    return nc


def _get_nc(bench_reps=None):
    key = bench_reps
    if key not in _CACHED:
        _CACHED[key] = _build(bench_reps)
    return _CACHED[key]


def _split16(x):
    hi = x.astype(np.float16)
    lo = (x.astype(np.float64) - hi.astype(np.float64)).astype(np.float16)
    return hi, lo


def make_in_maps(feat, mask, w_attn):
    feat2 = feat.reshape(B, C, N_FULL)
    mask2 = 2.0 * np.clip(mask.reshape(B, N_FULL).astype(np.float64), 0.0, 1.0)
    wh, wl = _split16(w_attn.astype(np.float32))
    whrep = np.ascontiguousarray(np.tile(wh[:, None], (1, 128)))
    wlrep = np.ascontiguousarray(np.tile(wl[:, None], (1, 128)))
    in_maps = []
    for core in range(N_CORES):
        b, q = divmod(core, Q_PER_B)
        fs = np.ascontiguousarray(feat2[b, :, q * NS:(q + 1) * NS], dtype=np.float32)
        m2 = mask2[b, q * NS:(q + 1) * NS]
        mh = m2.astype(np.float16)
        ml = (m2 - mh.astype(np.float64)).astype(np.float16)
        in_maps.append({
            "feat": fs,
            "mrows": np.ascontiguousarray(np.stack([mh, ml])),
            "whrep": whrep,
            "wlrep": wlrep,
        })
    return in_maps


def combine(results):
    out = np.zeros((B, C), dtype=np.float32)
    for b in range(B):
        v = np.zeros(C, dtype=np.float64)
        s = 0.0
        for q in range(Q_PER_B):
            r = results[b * Q_PER_B + q]["out_vs"]
            v += r[:, 0].astype(np.float64)
            s += float(r[0, 1])
        out[b] = (v / s).astype(np.float32)
    return out


def run_on_cores(feat, mask, w_attn, bench_reps=None):
    nc = _get_nc(bench_reps)
    in_maps = make_in_maps(np.asarray(feat), np.asarray(mask), np.asarray(w_attn))
    res = run_bass_kernel_spmd(nc, in_maps, core_ids=list(range(N_CORES)))
    return res


def kernel(feat, mask, w_attn):
    res = run_on_cores(feat, mask, w_attn)
    return combine(res.results)


# revision 5
# speedup vs baseline: 1.0213x; 1.0213x over previous
"""AttnPool3D Trainium2 kernel.

Reference computation (B=2, C=128, D=48, H=96, W=96, N = D*H*W = 442368):
    logits = einsum('bcdhw,c->bdhw', feat, w_attn) + 2.0*clip(mask, 0, 1)
    w = softmax(logits.reshape(B, -1), axis=-1)
    out = einsum('bcn,bn->bc', feat.reshape(B, C, -1), w)

Sharding: 8 cores = (batch b in 0..1) x (spatial quarter q in 0..3).
Each core processes feat[b, :, q*Ns:(q+1)*Ns] (Ns = 110592) in ONE pass over
HBM (single-pass softmax without a max subtraction; logits are bounded, a
constant bias -8 prevents overflow and cancels in the final normalization):

  per chunk of 2048 spatial columns (54 chunks):
    - DMA feat chunk A [128, 2048] fp32 (C on partitions)
    - Pool/DVE: cast A -> Ah fp16
    - PE (per 512-sub-chunk, accumulating into PSUM [128, 512]):
        X  = wh_rep^T @ Ah      (logits "high" part, broadcast to all parts)
        X += wl_rep^T @ Ah      (w = wh + wl fp16 split: w captured to 2^-22)
        X += ones2^T @ [mh; ml] (2*clip(mask) fp16-split rows, K=2 matmul)
    - ACT: Pb = exp(X - 8), accum_out -> s_chunk [128, 1]
    - DVE: stt junk = A * Pb (fp32 ALU), accum_out -> v_chunk [128, 1]
  reduce v/s over chunks -> out_vs [128, 2]

Host combines: out[b, c] = sum_q v / sum_q s (fp64; the -8 bias cancels).
"""
import sys

sys.path.insert(0, "/opt/trn_rl_repo")

import numpy as np

import concourse.bass as bass
import concourse.tile as tile
from concourse import mybir, bacc
from concourse.bass_utils import run_bass_kernel_spmd

B, C = 2, 128
N_FULL = 48 * 96 * 96          # 442368
N_CORES = 8
Q_PER_B = 4                    # spatial quarters per batch
NS = N_FULL // Q_PER_B         # 110592 per core
F_CHUNK = 2048                 # spatial columns per chunk
N_CHUNKS = NS // F_CHUNK       # 54
SUB = 512                      # matmul free dim (one PSUM bank fp32)
N_SUB = F_CHUNK // SUB         # 4
EXP_BIAS = -8.0
DVE_CAST_EVERY = 10**9         # all casts on Pool (DVE cast entangles the serial chain)

f32 = mybir.dt.float32
f16 = mybir.dt.float16

_CACHED = {}


def _build(bench_reps=None):
    """bench_reps=None -> production straight-line kernel.
    bench_reps=R -> same body wrapped in a For_i(R) repeat loop (for HW
    timing via wall-clock deltas between two R values)."""
    nc = bacc.Bacc("TRN2", target_bir_lowering=False, debug=False)

    feat_dram = nc.dram_tensor("feat", [C, NS], f32, kind="ExternalInput")
    mrows_dram = nc.dram_tensor("mrows", [2, NS], f16, kind="ExternalInput")
    whrep_dram = nc.dram_tensor("whrep", [C, 128], f16, kind="ExternalInput")
    wlrep_dram = nc.dram_tensor("wlrep", [C, 128], f16, kind="ExternalInput")
    out_dram = nc.dram_tensor("out_vs", [C, 2], f32, kind="ExternalOutput")

    with tile.TileContext(nc) as tc:
        with (
            tc.tile_pool(name="weights", bufs=1) as wpool,
            tc.tile_pool(name="feat", bufs=6) as apool,
            tc.tile_pool(name="feat16", bufs=4) as ahpool,
            tc.tile_pool(name="mask", bufs=8) as mpool,
            tc.tile_pool(name="prob", bufs=4) as ppool,
            tc.tile_pool(name="junk", bufs=3) as jpool,
            tc.tile_pool(name="accs", bufs=1) as accpool,
            tc.tile_pool(name="psum", bufs=2, space="PSUM") as psum,
        ):
            whrep = wpool.tile([C, 128], f16)
            wlrep = wpool.tile([C, 128], f16)
            nc.sync.dma_start(whrep[:], whrep_dram.ap())
            nc.sync.dma_start(wlrep[:], wlrep_dram.ap())
            ones2 = wpool.tile([2, 128], f16)
            nc.vector.memset(ones2[:], 1.0)
            bias_t = wpool.tile([C, 1], f32)
            nc.vector.memset(bias_t[:], EXP_BIAS)

            v_accs = accpool.tile([C, N_CHUNKS], f32)
            s_accs = accpool.tile([C, N_CHUNKS], f32)

            def emit_chunk(ci):
                a = apool.tile([C, F_CHUNK], f32, tag="a")
                nc.sync.dma_start(
                    a[:], feat_dram.ap()[:, ci * F_CHUNK:(ci + 1) * F_CHUNK])
                mrows = mpool.tile([2, F_CHUNK], f16, tag="mrows")
                nc.sync.dma_start(
                    mrows[:], mrows_dram.ap()[:, ci * F_CHUNK:(ci + 1) * F_CHUNK])

                ah = ahpool.tile([C, F_CHUNK], f16, tag="ah")
                if ci % DVE_CAST_EVERY == DVE_CAST_EVERY - 1:
                    nc.vector.tensor_copy(ah[:], a[:])
                else:
                    nc.gpsimd.tensor_copy(ah[:], a[:])

                x = psum.tile([C, F_CHUNK], f32, tag="x")
                # group same-weight matmuls to minimize weight switching
                for si in range(N_SUB):
                    nc.tensor.matmul(x[:, si * SUB:(si + 1) * SUB], whrep[:],
                                     ah[:, si * SUB:(si + 1) * SUB],
                                     start=True, stop=False)
                for si in range(N_SUB):
                    nc.tensor.matmul(x[:, si * SUB:(si + 1) * SUB], wlrep[:],
                                     ah[:, si * SUB:(si + 1) * SUB],
                                     start=False, stop=False)
                for si in range(N_SUB):
                    nc.tensor.matmul(x[:, si * SUB:(si + 1) * SUB], ones2[:],
                                     mrows[:, si * SUB:(si + 1) * SUB],
                                     start=False, stop=True)

                pb = ppool.tile([C, F_CHUNK], f32, tag="pb")
                nc.scalar.activation(
                    pb[:], x[:], mybir.ActivationFunctionType.Exp,
                    bias=bias_t[:], scale=1.0,
                    accum_out=s_accs[:, ci:ci + 1],
                )

                junk = jpool.tile([C, F_CHUNK], f32, tag="junk")
                nc.vector.scalar_tensor_tensor(
                    junk[:], a[:], 1.0, pb[:],
                    op0=mybir.AluOpType.mult, op1=mybir.AluOpType.mult,
                    accum_out=v_accs[:, ci:ci + 1],
                )

            if bench_reps is None:
                for ci in range(N_CHUNKS):
                    emit_chunk(ci)
            else:
                with tc.For_i(0, bench_reps, 1,
                              hint_engines=(mybir.EngineType.PE,)):
                    for ci in range(N_CHUNKS):
                        emit_chunk(ci)

            out_sb = accpool.tile([C, 2], f32)
            nc.vector.reduce_sum(out_sb[:, 0:1], v_accs[:], axis=mybir.AxisListType.X)
            nc.vector.reduce_sum(out_sb[:, 1:2], s_accs[:], axis=mybir.AxisListType.X)
            nc.sync.dma_start(out_dram.ap(), out_sb[:])

    nc.compile()
    return nc


def _get_nc(bench_reps=None):
    key = bench_reps
    if key not in _CACHED:
        _CACHED[key] = _build(bench_reps)
    return _CACHED[key]


def _split16(x):
    hi = x.astype(np.float16)
    lo = (x.astype(np.float64) - hi.astype(np.float64)).astype(np.float16)
    return hi, lo


def make_in_maps(feat, mask, w_attn):
    feat2 = feat.reshape(B, C, N_FULL)
    mask2 = 2.0 * np.clip(mask.reshape(B, N_FULL).astype(np.float64), 0.0, 1.0)
    wh, wl = _split16(w_attn.astype(np.float32))
    whrep = np.ascontiguousarray(np.tile(wh[:, None], (1, 128)))
    wlrep = np.ascontiguousarray(np.tile(wl[:, None], (1, 128)))
    in_maps = []
    for core in range(N_CORES):
        b, q = divmod(core, Q_PER_B)
        fs = np.ascontiguousarray(feat2[b, :, q * NS:(q + 1) * NS], dtype=np.float32)
        m2 = mask2[b, q * NS:(q + 1) * NS]
        mh = m2.astype(np.float16)
        ml = (m2 - mh.astype(np.float64)).astype(np.float16)
        in_maps.append({
            "feat": fs,
            "mrows": np.ascontiguousarray(np.stack([mh, ml])),
            "whrep": whrep,
            "wlrep": wlrep,
        })
    return in_maps


def combine(results):
    out = np.zeros((B, C), dtype=np.float32)
    for b in range(B):
        v = np.zeros(C, dtype=np.float64)
        s = 0.0
        for q in range(Q_PER_B):
            r = results[b * Q_PER_B + q]["out_vs"]
            v += r[:, 0].astype(np.float64)
            s += float(r[0, 1])
        out[b] = (v / s).astype(np.float32)
    return out


def run_on_cores(feat, mask, w_attn, bench_reps=None):
    nc = _get_nc(bench_reps)
    in_maps = make_in_maps(np.asarray(feat), np.asarray(mask), np.asarray(w_attn))
    res = run_bass_kernel_spmd(nc, in_maps, core_ids=list(range(N_CORES)))
    return res


def kernel(feat, mask, w_attn):
    res = run_on_cores(feat, mask, w_attn)
    return combine(res.results)


# revision 9
# speedup vs baseline: 1.4004x; 1.3711x over previous
"""AttnPool3D Trainium2 kernel.

Reference computation (B=2, C=128, D=48, H=96, W=96, N = D*H*W = 442368):
    logits = einsum('bcdhw,c->bdhw', feat, w_attn) + 2.0*clip(mask, 0, 1)
    w = softmax(logits.reshape(B, -1), axis=-1)
    out = einsum('bcn,bn->bc', feat.reshape(B, C, -1), w)

Sharding: 8 cores = (batch b in 0..1) x (spatial quarter q in 0..3).
Each core processes feat[b, :, q*Ns:(q+1)*Ns] (Ns = 110592) in ONE pass over
HBM (single-pass softmax without a max subtraction; logits are bounded, a
constant bias -8 prevents overflow and cancels in the final normalization):

  per chunk of 2048 spatial columns (54 chunks):
    - DMA feat chunk A [128, 2048] fp32 (C on partitions)
    - Pool/DVE: cast A -> Ah fp16
    - PE (per 512-sub-chunk, accumulating into PSUM [128, 512]):
        X  = wh_rep^T @ Ah      (logits "high" part, broadcast to all parts)
        X += wl_rep^T @ Ah      (w = wh + wl fp16 split: w captured to 2^-22)
        X += ones2^T @ [mh; ml] (2*clip(mask) fp16-split rows, K=2 matmul)
    - ACT: Pb = exp(X - 8), accum_out -> s_chunk [128, 1]
    - DVE: stt junk = A * Pb (fp32 ALU), accum_out -> v_chunk [128, 1]
  reduce v/s over chunks -> out_vs [128, 2]

Host combines: out[b, c] = sum_q v / sum_q s (fp64; the -8 bias cancels).
"""
import sys

sys.path.insert(0, "/opt/trn_rl_repo")

import numpy as np

import concourse.bass as bass
import concourse.tile as tile
from concourse import mybir, bacc
from concourse.bass_utils import run_bass_kernel_spmd

B, C = 2, 128
N_FULL = 48 * 96 * 96          # 442368
N_CORES = 8
Q_PER_B = 4                    # spatial quarters per batch
NS = N_FULL // Q_PER_B         # 110592 per core
F_CHUNK = 2048                 # spatial columns per chunk
N_CHUNKS = NS // F_CHUNK       # 54
SUB = 512                      # matmul free dim (one PSUM bank fp32)
N_SUB = F_CHUNK // SUB         # 4
EXP_BIAS = -8.0
DVE_CAST_EVERY = 10**9         # all casts on Pool (DVE cast entangles the serial chain)

f32 = mybir.dt.float32
f16 = mybir.dt.float16

_CACHED = {}


def _build(bench_reps=None, variant="full"):
    """bench_reps=None -> production straight-line kernel.
    bench_reps=R -> same body wrapped in a For_i(R) repeat loop (for HW
    timing via wall-clock deltas between two R values).
    variant: ablation selector for bottleneck hunting ("full", "nocast",
    "nostt", "nomm", "noexp", "dmaonly")."""
    nc = bacc.Bacc("TRN2", target_bir_lowering=False, debug=False)

    feat_dram = nc.dram_tensor("feat", [C, NS], f32, kind="ExternalInput")
    mrows_dram = nc.dram_tensor("mrows", [2, NS], f16, kind="ExternalInput")
    whrep_dram = nc.dram_tensor("whrep", [C, 128], f16, kind="ExternalInput")
    wlrep_dram = nc.dram_tensor("wlrep", [C, 128], f16, kind="ExternalInput")
    out_dram = nc.dram_tensor("out_vs", [C, 2], f32, kind="ExternalOutput")

    with tile.TileContext(nc) as tc:
        with (
            tc.tile_pool(name="weights", bufs=1) as wpool,
            tc.tile_pool(name="feat", bufs=6) as apool,
            tc.tile_pool(name="feat16", bufs=4) as ahpool,
            tc.tile_pool(name="mask", bufs=8) as mpool,
            tc.tile_pool(name="prob", bufs=4) as ppool,
            tc.tile_pool(name="junk", bufs=3) as jpool,
            tc.tile_pool(name="accs", bufs=1) as accpool,
            tc.tile_pool(name="psum", bufs=2, space="PSUM") as psum,
        ):
            whrep = wpool.tile([C, 128], f16)
            wlrep = wpool.tile([C, 128], f16)
            nc.sync.dma_start(whrep[:], whrep_dram.ap())
            nc.sync.dma_start(wlrep[:], wlrep_dram.ap())
            ones2 = wpool.tile([2, 128], f16)
            nc.vector.memset(ones2[:], 1.0)
            bias_t = wpool.tile([C, 1], f32)
            nc.vector.memset(bias_t[:], EXP_BIAS)

            v_accs = accpool.tile([C, N_CHUNKS], f32)
            s_accs = accpool.tile([C, N_CHUNKS], f32)

            ah_const = None
            if variant == "nocast":
                ah_const = wpool.tile([C, F_CHUNK], f16)
                nc.vector.memset(ah_const[:], 0.001)

            def emit_chunk(ci):
                a = apool.tile([C, F_CHUNK], f32, tag="a")
                nc.sync.dma_start(
                    a[:], feat_dram.ap()[:, ci * F_CHUNK:(ci + 1) * F_CHUNK])
                mrows = mpool.tile([2, F_CHUNK], f16, tag="mrows")
                nc.sync.dma_start(
                    mrows[:], mrows_dram.ap()[:, ci * F_CHUNK:(ci + 1) * F_CHUNK])
                if variant == "dmaonly":
                    return

                if variant == "nocast":
                    ah = ah_const
                else:
                    ah = ahpool.tile([C, F_CHUNK], f16, tag="ah")
                    if ci % DVE_CAST_EVERY == DVE_CAST_EVERY - 1:
                        nc.vector.tensor_copy(ah[:], a[:])
                    else:
                        nc.gpsimd.tensor_copy(ah[:], a[:])

                x = psum.tile([C, F_CHUNK], f32, tag="x")
                if variant != "nomm":
                    # group same-weight matmuls to minimize weight switching
                    for si in range(N_SUB):
                        nc.tensor.matmul(x[:, si * SUB:(si + 1) * SUB], whrep[:],
                                         ah[:, si * SUB:(si + 1) * SUB],
                                         start=True, stop=False)
                    for si in range(N_SUB):
                        nc.tensor.matmul(x[:, si * SUB:(si + 1) * SUB], wlrep[:],
                                         ah[:, si * SUB:(si + 1) * SUB],
                                         start=False, stop=False)
                    for si in range(N_SUB):
                        nc.tensor.matmul(x[:, si * SUB:(si + 1) * SUB], ones2[:],
                                         mrows[:, si * SUB:(si + 1) * SUB],
                                         start=False, stop=True)

                pb = ppool.tile([C, F_CHUNK], f32, tag="pb")
                if variant != "noexp":
                    nc.scalar.activation(
                        pb[:], x[:],
                        mybir.ActivationFunctionType.Exp,
                        bias=bias_t[:], scale=1.0,
                        accum_out=s_accs[:, ci:ci + 1],
                    )

                if variant != "nostt":
                    junk = jpool.tile([C, F_CHUNK], f32, tag="junk")
                    nc.vector.scalar_tensor_tensor(
                        junk[:], a[:], 1.0, pb[:],
                        op0=mybir.AluOpType.mult, op1=mybir.AluOpType.mult,
                        accum_out=v_accs[:, ci:ci + 1],
                    )

            if bench_reps is None:
                for ci in range(N_CHUNKS):
                    emit_chunk(ci)
            else:
                with tc.For_i(0, bench_reps, 1,
                              hint_engines=(mybir.EngineType.PE,)):
                    for ci in range(N_CHUNKS):
                        emit_chunk(ci)

            out_sb = accpool.tile([C, 2], f32)
            nc.vector.reduce_sum(out_sb[:, 0:1], v_accs[:], axis=mybir.AxisListType.X)
            nc.vector.reduce_sum(out_sb[:, 1:2], s_accs[:], axis=mybir.AxisListType.X)
            nc.sync.dma_start(out_dram.ap(), out_sb[:])

    nc.compile()
    return nc


def _get_nc(bench_reps=None, variant="full"):
    key = (bench_reps, variant)
    if key not in _CACHED:
        _CACHED[key] = _build(bench_reps, variant)
    return _CACHED[key]


def _split16(x):
    hi = x.astype(np.float16)
    lo = (x.astype(np.float64) - hi.astype(np.float64)).astype(np.float16)
    return hi, lo


def make_in_maps(feat, mask, w_attn):
    feat2 = feat.reshape(B, C, N_FULL)
    mask2 = 2.0 * np.clip(mask.reshape(B, N_FULL).astype(np.float64), 0.0, 1.0)
    wh, wl = _split16(w_attn.astype(np.float32))
    whrep = np.ascontiguousarray(np.tile(wh[:, None], (1, 128)))
    wlrep = np.ascontiguousarray(np.tile(wl[:, None], (1, 128)))
    in_maps = []
    for core in range(N_CORES):
        b, q = divmod(core, Q_PER_B)
        fs = np.ascontiguousarray(feat2[b, :, q * NS:(q + 1) * NS], dtype=np.float32)
        m2 = mask2[b, q * NS:(q + 1) * NS]
        mh = m2.astype(np.float16)
        ml = (m2 - mh.astype(np.float64)).astype(np.float16)
        in_maps.append({
            "feat": fs,
            "mrows": np.ascontiguousarray(np.stack([mh, ml])),
            "whrep": whrep,
            "wlrep": wlrep,
        })
    return in_maps


def combine(results):
    out = np.zeros((B, C), dtype=np.float32)
    for b in range(B):
        v = np.zeros(C, dtype=np.float64)
        s = 0.0
        for q in range(Q_PER_B):
            r = results[b * Q_PER_B + q]["out_vs"]
            v += r[:, 0].astype(np.float64)
            s += float(r[0, 1])
        out[b] = (v / s).astype(np.float32)
    return out


def run_on_cores(feat, mask, w_attn, bench_reps=None):
    nc = _get_nc(bench_reps)
    in_maps = make_in_maps(np.asarray(feat), np.asarray(mask), np.asarray(w_attn))
    res = run_bass_kernel_spmd(nc, in_maps, core_ids=list(range(N_CORES)))
    return res


def kernel(feat, mask, w_attn):
    res = run_on_cores(feat, mask, w_attn)
    return combine(res.results)


# revision 10
# speedup vs baseline: 3.2901x; 2.3495x over previous
"""AttnPool3D Trainium2 kernel.

Reference computation (B=2, C=128, D=48, H=96, W=96, N = D*H*W = 442368):
    logits = einsum('bcdhw,c->bdhw', feat, w_attn) + 2.0*clip(mask, 0, 1)
    w = softmax(logits.reshape(B, -1), axis=-1)
    out = einsum('bcn,bn->bc', feat.reshape(B, C, -1), w)

Sharding: 8 cores = (batch b in 0..1) x (spatial quarter q in 0..3).
Each core processes feat[b, :, q*Ns:(q+1)*Ns] (Ns = 110592) in ONE pass over
HBM (single-pass softmax without a max subtraction; logits are bounded, a
constant bias -8 prevents overflow and cancels in the final normalization):

  per chunk of 2048 spatial columns (54 chunks):
    - DMA feat chunk A [128, 2048] fp32 (C on partitions)
    - Pool/DVE: cast A -> Ah fp16
    - PE (per 512-sub-chunk, accumulating into PSUM [128, 512]):
        X  = wh_rep^T @ Ah      (logits "high" part, broadcast to all parts)
        X += wl_rep^T @ Ah      (w = wh + wl fp16 split: w captured to 2^-22)
        X += ones2^T @ [mh; ml] (2*clip(mask) fp16-split rows, K=2 matmul)
    - ACT: Pb = exp(X - 8), accum_out -> s_chunk [128, 1]
    - DVE: stt junk = A * Pb (fp32 ALU), accum_out -> v_chunk [128, 1]
  reduce v/s over chunks -> out_vs [128, 2]

Host combines: out[b, c] = sum_q v / sum_q s (fp64; the -8 bias cancels).
"""
import sys

sys.path.insert(0, "/opt/trn_rl_repo")

import numpy as np

import concourse.bass as bass
import concourse.tile as tile
from concourse import mybir, bacc
from concourse.bass_utils import run_bass_kernel_spmd

B, C = 2, 128
N_FULL = 48 * 96 * 96          # 442368
N_CORES = 8
Q_PER_B = 4                    # spatial quarters per batch
NS = N_FULL // Q_PER_B         # 110592 per core
F_CHUNK = 2048                 # spatial columns per chunk
N_CHUNKS = NS // F_CHUNK       # 54
SUB = 512                      # matmul free dim (one PSUM bank fp32)
N_SUB = F_CHUNK // SUB         # 4
EXP_BIAS = -8.0
DVE_CAST_EVERY = 2             # alternate casts Pool/DVE (stt is lag-emitted)
STT_LAG = 2                    # chunks of lag between exp and its stt emission

f32 = mybir.dt.float32
f16 = mybir.dt.float16

_CACHED = {}


def _build(bench_reps=None, variant="full"):
    """bench_reps=None -> production straight-line kernel.
    bench_reps=R -> same body wrapped in a For_i(R) repeat loop (for HW
    timing via wall-clock deltas between two R values).
    variant: ablation selector for bottleneck hunting ("full", "nocast",
    "nostt", "nomm", "noexp", "dmaonly")."""
    nc = bacc.Bacc("TRN2", target_bir_lowering=False, debug=False)

    feat_dram = nc.dram_tensor("feat", [C, NS], f32, kind="ExternalInput")
    mrows_dram = nc.dram_tensor("mrows", [2, NS], f16, kind="ExternalInput")
    whrep_dram = nc.dram_tensor("whrep", [C, 128], f16, kind="ExternalInput")
    out_dram = nc.dram_tensor("out_vs", [C, 2], f32, kind="ExternalOutput")

    with tile.TileContext(nc) as tc:
        with (
            tc.tile_pool(name="weights", bufs=1) as wpool,
            tc.tile_pool(name="feat", bufs=8) as apool,
            tc.tile_pool(name="feat16", bufs=4) as ahpool,
            tc.tile_pool(name="mask", bufs=8) as mpool,
            tc.tile_pool(name="prob", bufs=6) as ppool,
            tc.tile_pool(name="junk", bufs=3) as jpool,
            tc.tile_pool(name="accs", bufs=1) as accpool,
            tc.tile_pool(name="psum", bufs=2, space="PSUM") as psum,
        ):
            whrep = wpool.tile([C, 128], f16)
            nc.sync.dma_start(whrep[:], whrep_dram.ap())
            ones2 = wpool.tile([2, 128], f16)
            nc.vector.memset(ones2[:], 1.0)
            bias_t = wpool.tile([C, 1], f32)
            nc.vector.memset(bias_t[:], EXP_BIAS)

            v_accs = accpool.tile([C, N_CHUNKS], f32)
            s_accs = accpool.tile([C, N_CHUNKS], f32)
            if variant in ("dmaonly", "nostt", "noexp"):
                nc.vector.memset(v_accs[:], 1.0)
                nc.vector.memset(s_accs[:], 1.0)

            ah_const = None
            if variant == "nocast":
                ah_const = wpool.tile([C, F_CHUNK], f16)
                nc.vector.memset(ah_const[:], 0.001)

            def emit_chunk(ci):
                a = apool.tile([C, F_CHUNK], f32, tag="a")
                nc.sync.dma_start(
                    a[:], feat_dram.ap()[:, ci * F_CHUNK:(ci + 1) * F_CHUNK])
                mrows = mpool.tile([2, F_CHUNK], f16, tag="mrows")
                nc.sync.dma_start(
                    mrows[:], mrows_dram.ap()[:, ci * F_CHUNK:(ci + 1) * F_CHUNK])
                if variant == "dmaonly":
                    return

                if variant == "nocast":
                    ah = ah_const
                else:
                    ah = ahpool.tile([C, F_CHUNK], f16, tag="ah")
                    if ci % DVE_CAST_EVERY == DVE_CAST_EVERY - 1:
                        nc.vector.tensor_copy(ah[:], a[:])
                    else:
                        nc.gpsimd.tensor_copy(ah[:], a[:])

                x = psum.tile([C, F_CHUNK], f32, tag="x")
                if variant != "nomm":
                    # group same-weight matmuls to minimize weight switching
                    for si in range(N_SUB):
                        nc.tensor.matmul(x[:, si * SUB:(si + 1) * SUB], whrep[:],
                                         ah[:, si * SUB:(si + 1) * SUB],
                                         start=True, stop=False)
                    for si in range(N_SUB):
                        nc.tensor.matmul(x[:, si * SUB:(si + 1) * SUB], ones2[:],
                                         mrows[:, si * SUB:(si + 1) * SUB],
                                         start=False, stop=True)

                pb = ppool.tile([C, F_CHUNK], f32, tag="pb")
                if variant != "noexp":
                    nc.scalar.activation(
                        pb[:], x[:],
                        mybir.ActivationFunctionType.Exp,
                        bias=bias_t[:], scale=1.0,
                        accum_out=s_accs[:, ci:ci + 1],
                    )
                return a, pb

            def emit_stt(ci, a, pb):
                if variant in ("nostt", "dmaonly"):
                    return
                junk = jpool.tile([C, F_CHUNK], f32, tag="junk")
                nc.vector.scalar_tensor_tensor(
                    junk[:], a[:], 1.0, pb[:],
                    op0=mybir.AluOpType.mult, op1=mybir.AluOpType.mult,
                    accum_out=v_accs[:, ci:ci + 1],
                )

            def emit_all():
                # stt for chunk ci is emitted LAG chunks later so the DVE
                # cast of chunk ci+1 is not stuck behind stt(ci) in DVE
                # program order (the stt's inputs are long-ready by then).
                pending = []
                for ci in range(N_CHUNKS):
                    r = emit_chunk(ci)
                    if r is not None:
                        pending.append((ci, *r))
                    while len(pending) > STT_LAG:
                        emit_stt(*pending.pop(0))
                while pending:
                    emit_stt(*pending.pop(0))

            if bench_reps is None:
                emit_all()
            else:
                with tc.For_i(0, bench_reps, 1,
                              hint_engines=(mybir.EngineType.PE,)):
                    emit_all()

            out_sb = accpool.tile([C, 2], f32)
            nc.vector.reduce_sum(out_sb[:, 0:1], v_accs[:], axis=mybir.AxisListType.X)
            nc.vector.reduce_sum(out_sb[:, 1:2], s_accs[:], axis=mybir.AxisListType.X)
            nc.sync.dma_start(out_dram.ap(), out_sb[:])

    nc.compile()
    return nc


def _get_nc(bench_reps=None, variant="full"):
    key = (bench_reps, variant)
    if key not in _CACHED:
        _CACHED[key] = _build(bench_reps, variant)
    return _CACHED[key]


def _split16(x):
    hi = x.astype(np.float16)
    lo = (x.astype(np.float64) - hi.astype(np.float64)).astype(np.float16)
    return hi, lo


def make_in_maps(feat, mask, w_attn):
    feat2 = feat.reshape(B, C, N_FULL)
    mask2 = 2.0 * np.clip(mask.reshape(B, N_FULL).astype(np.float64), 0.0, 1.0)
    wh = w_attn.astype(np.float32).astype(np.float16)
    whrep = np.ascontiguousarray(np.tile(wh[:, None], (1, 128)))
    in_maps = []
    for core in range(N_CORES):
        b, q = divmod(core, Q_PER_B)
        fs = np.ascontiguousarray(feat2[b, :, q * NS:(q + 1) * NS], dtype=np.float32)
        m2 = mask2[b, q * NS:(q + 1) * NS]
        mh = m2.astype(np.float16)
        ml = (m2 - mh.astype(np.float64)).astype(np.float16)
        in_maps.append({
            "feat": fs,
            "mrows": np.ascontiguousarray(np.stack([mh, ml])),
            "whrep": whrep,
        })
    return in_maps


def combine(results):
    out = np.zeros((B, C), dtype=np.float32)
    for b in range(B):
        v = np.zeros(C, dtype=np.float64)
        s = 0.0
        for q in range(Q_PER_B):
            r = results[b * Q_PER_B + q]["out_vs"]
            v += r[:, 0].astype(np.float64)
            s += float(r[0, 1])
        out[b] = (v / s).astype(np.float32)
    return out


def run_on_cores(feat, mask, w_attn, bench_reps=None):
    nc = _get_nc(bench_reps)
    in_maps = make_in_maps(np.asarray(feat), np.asarray(mask), np.asarray(w_attn))
    res = run_bass_kernel_spmd(nc, in_maps, core_ids=list(range(N_CORES)))
    return res


def kernel(feat, mask, w_attn):
    res = run_on_cores(feat, mask, w_attn)
    return combine(res.results)
